# revision 10
# baseline (speedup 1.0000x reference)
"""MoE router gate (DeepSeek-V3 style) on 8 Trainium2 NeuronCores.

Math (per token):
  logits = x @ w.T            [N=16384, E=256], D=7168, fp32
  scores = sigmoid(logits)
  s      = scores + bias
  group top-2 sums over 8 groups of 32 -> keep top-4 groups
  indices = top-8 of s within kept groups
  weights = renormalize(scores[indices]) * 2.5

Sharding: data-parallel over tokens (2048/core); w+bias replicated.

GEMM strategy: fp16 3-pass split for near-fp32 precision at 1 cy/row:
  x = xh + xl*2^-11, w = wh + wl*2^-11   (host-side split, fp16 halves)
  logits ~= xh@wh + 2^-11*(xh@wl + xl@wh)   (xl@wl term ~2^-22, dropped)
Host also pre-transposes x to d-major so both matmul operands stream
naturally (contraction dim on partitions).
"""

import sys
import os
import threading

sys.path.insert(0, "/opt/trn_rl_repo")

import numpy as np

import concourse.bass as bass
import concourse.bacc as bacc
import concourse.mybir as mybir
import concourse.tile as tile
from concourse.bass_utils import run_bass_kernel_spmd

N_TOK = 16384
D = 7168
E = 256
N_CORES = 8
NSH = N_TOK // N_CORES          # tokens per core
TOK_TILE = 128
N_TILES = NSH // TOK_TILE       # 16
KC = 128                        # contraction chunk
N_KC = D // KC                  # 56
N_GROUPS = 8
GSIZE = E // N_GROUPS           # 32
TOPK_GROUPS = 4
TOPK = 8
ROUTE_SCALE = 2.5
SPLIT_SCALE = 2048.0            # 2^11
NEG_BIG = 1.0e30

_cached = {}


def _build_nc():
    """Per-core bass program. SPMD: same program, per-core input maps."""
    fp16 = mybir.dt.float16
    f32 = mybir.dt.float32
    u32 = mybir.dt.uint32

    nc = bacc.Bacc(trn_type="TRN2", target_bir_lowering=False)

    xh_d = nc.dram_tensor("xh", [D, NSH], fp16, kind="ExternalInput")
    xl_d = nc.dram_tensor("xl", [D, NSH], fp16, kind="ExternalInput")
    # w packed [D, 512]: cols 0:256 = wh, 256:512 = wl (both fp16, wl scaled)
    w_d = nc.dram_tensor("w", [D, 2 * E], fp16, kind="ExternalInput")
    bias_d = nc.dram_tensor("bias", [128, E], f32, kind="ExternalInput")
    wts_d = nc.dram_tensor("wts", [NSH, TOPK], f32, kind="ExternalOutput")
    idx_d = nc.dram_tensor("idx", [NSH, TOPK], mybir.dt.int32, kind="ExternalOutput")

    with tile.TileContext(nc) as tc:
        with (
            tc.tile_pool(name="wpool", bufs=1) as wpool,
            tc.tile_pool(name="xpool", bufs=2) as xpool,
            tc.tile_pool(name="spool", bufs=2) as spool,
            tc.tile_pool(name="tiny", bufs=2) as tiny,
            tc.tile_pool(name="psum", bufs=2, space="PSUM") as pspool,
            tc.tile_pool(name="psum2", bufs=2, space="PSUM") as pspool2,
        ):
            # --- resident weights / bias ---
            # W and x are loaded in 8-chunk groups, each its own tile, so
            # dependencies are group-granular: the chunk-0 matmuls start as
            # soon as the first ~1.5MB lands instead of after the full
            # 11MB preload.
            WSUB = 8
            NG = N_KC // WSUB  # 7 groups
            wsb_g = []
            for g in range(NG):
                wg = wpool.tile([128, WSUB, 2 * E], fp16, tag=f"w{g}", bufs=1)
                nc.sync.dma_start(
                    wg[:, :, :],
                    w_d[g * WSUB * 128 : (g + 1) * WSUB * 128, :].rearrange(
                        "(c p) e -> p c e", p=128
                    ),
                )
                wsb_g.append(wg)
            bias_sb = wpool.tile([128, E], f32, tag="bias")
            nc.scalar.dma_start(bias_sb[:, :], bias_d[:, :])

            # x loads batched 2 token-tiles per DMA (512B contiguous runs)
            TOK_BLOCK = 2 * TOK_TILE
            xh_g = xl_g = None
            for t in range(N_TILES):
                ts = t * TOK_TILE
                sub = t % 2
                if sub == 0:
                    bs = t * TOK_TILE
                    # descriptor generation serializes per issuing engine
                    # (~3us per 1024-row dma_start), so xh goes on the
                    # scalar sequencer and xl on gpsimd, in parallel with
                    # W-group generation on sync.
                    xh_g, xl_g = [], []
                    for g in range(NG):
                        r0, r1 = g * WSUB * 128, (g + 1) * WSUB * 128
                        xhg = xpool.tile(
                            [128, WSUB, TOK_BLOCK], fp16, tag=f"xh{g}", bufs=2
                        )
                        nc.scalar.dma_start(
                            xhg[:, :, :],
                            xh_d[r0:r1, bs : bs + TOK_BLOCK].rearrange(
                                "(c p) n -> p c n", p=128
                            ),
                        )
                        xh_g.append(xhg)
                        xlg = xpool.tile(
                            [128, WSUB, TOK_BLOCK], fp16, tag=f"xl{g}", bufs=2
                        )
                        nc.gpsimd.dma_start(
                            xlg[:, :, :],
                            xl_d[r0:r1, bs : bs + TOK_BLOCK].rearrange(
                                "(c p) n -> p c n", p=128
                            ),
                        )
                        xl_g.append(xlg)

                tsl = slice(sub * TOK_TILE, (sub + 1) * TOK_TILE)
                ps1 = pspool.tile([128, 2 * E], f32, tag="ps1")
                ps2 = pspool2.tile([128, E], f32, tag="ps2")
                for c in range(N_KC):
                    g, ci = c // WSUB, c % WSUB
                    nc.tensor.matmul(
                        ps1[:, :],
                        xh_g[g][:, ci, tsl],
                        wsb_g[g][:, ci, :],
                        start=(c == 0),
                        stop=(c == N_KC - 1),
                    )
                    nc.tensor.matmul(
                        ps2[:, :],
                        xl_g[g][:, ci, tsl],
                        wsb_g[g][:, ci, 0:E],
                        start=(c == 0),
                        stop=(c == N_KC - 1),
                    )

                # logits = ps1[:, :E] + 2^-11 * (ps1[:, E:] + ps2)
                t2 = spool.tile([128, E], f32, tag="t2")
                nc.scalar.activation(
                    t2[:, :], ps2[:, :], mybir.ActivationFunctionType.Copy,
                    scale=1.0 / SPLIT_SCALE,
                )
                u = spool.tile([128, E], f32, tag="u")
                nc.vector.scalar_tensor_tensor(
                    u[:, :], ps1[:, E:], 1.0 / SPLIT_SCALE, t2[:, :],
                    op0=mybir.AluOpType.mult, op1=mybir.AluOpType.add,
                )
                logits = spool.tile([128, E], f32, tag="logits")
                nc.vector.tensor_add(logits[:, :], u[:, :], ps1[:, 0:E])

                # scores = sigmoid(logits); s = scores + bias
                scores = spool.tile([128, E], f32, tag="scores")
                nc.scalar.activation(
                    scores[:, :], logits[:, :], mybir.ActivationFunctionType.Sigmoid
                )
                s = spool.tile([128, E], f32, tag="s")
                nc.vector.tensor_add(s[:, :], scores[:, :], bias_sb[:, :])

                # group top-2 sums
                gtop = tiny.tile([128, N_GROUPS, 8], f32, tag="gtop")
                for g in range(N_GROUPS):
                    nc.vector.max(gtop[:, g, :], s[:, g * GSIZE : (g + 1) * GSIZE])
                gs = tiny.tile([128, N_GROUPS], f32, tag="gs")
                nc.vector.tensor_add(gs[:, :], gtop[:, :, 0], gtop[:, :, 1])

                gsort = tiny.tile([128, 8], f32, tag="gsort")
                nc.vector.max(gsort[:, :], gs[:, :])
                keep = tiny.tile([128, N_GROUPS], f32, tag="keep")
                nc.vector.tensor_scalar(
                    keep[:, :], gs[:, :], gsort[:, 3:4], None,
                    op0=mybir.AluOpType.is_ge,
                )
                amask = tiny.tile([128, N_GROUPS], f32, tag="amask")
                nc.vector.tensor_scalar(
                    amask[:, :], keep[:, :], 1.0, NEG_BIG,
                    op0=mybir.AluOpType.subtract, op1=mybir.AluOpType.mult,
                )

                smask = spool.tile([128, N_GROUPS, GSIZE], f32, tag="smask")
                for g in range(N_GROUPS):
                    nc.vector.tensor_scalar(
                        smask[:, g, :], s[:, g * GSIZE : (g + 1) * GSIZE],
                        amask[:, g : g + 1], None, op0=mybir.AluOpType.add,
                    )

                smask2 = smask[:, :, :].rearrange("p g e -> p (g e)")
                top8v = tiny.tile([128, TOPK], f32, tag="top8v")
                nc.vector.max(top8v[:, :], smask2)
                top8i = tiny.tile([128, TOPK], u32, tag="top8i")
                nc.vector.max_index(top8i[:, :], top8v[:, :], smask2)

                # extract scores at selected positions, aligned to top8v order
                wsel = tiny.tile([128, TOPK], f32, tag="wsel")
                scratch = spool.tile([128, E], f32, tag="scratch")
                for j in range(TOPK):
                    nc.vector.scalar_tensor_tensor(
                        scratch[:, :], smask2, top8v[:, j : j + 1], scores[:, :],
                        op0=mybir.AluOpType.is_equal, op1=mybir.AluOpType.mult,
                        accum_out=wsel[:, j : j + 1],
                    )

                ssum = tiny.tile([128, 1], f32, tag="ssum")
                nc.vector.reduce_sum(ssum[:, :], wsel[:, :], axis=mybir.AxisListType.X)
                rec = tiny.tile([128, 1], f32, tag="rec")
                nc.vector.reciprocal(rec[:, :], ssum[:, :])
                wout = tiny.tile([128, TOPK], f32, tag="wout")
                nc.vector.tensor_scalar(
                    wout[:, :], wsel[:, :], rec[:, 0:1], ROUTE_SCALE,
                    op0=mybir.AluOpType.mult, op1=mybir.AluOpType.mult,
                )

                nc.sync.dma_start(wts_d[ts : ts + TOK_TILE, :], wout[:, :])
                nc.sync.dma_start(
                    idx_d[ts : ts + TOK_TILE, :],
                    top8i[:, :].bitcast(mybir.dt.int32),
                )
    nc.finalize()
    return nc


def _host_prep(x, weight, bias):
    """Split to fp16 hi/lo and transpose to d-major, per-core shards."""
    x = np.asarray(x, dtype=np.float32)
    weight = np.asarray(weight, dtype=np.float32)
    bias = np.asarray(bias, dtype=np.float32)

    wh = weight.astype(np.float16)
    wl = ((weight - wh.astype(np.float32)) * SPLIT_SCALE).astype(np.float16)
    w_packed = np.empty((D, 2 * E), dtype=np.float16)
    w_packed[:, :E] = wh.T
    w_packed[:, E:] = wl.T
    bias_rep = np.ascontiguousarray(np.broadcast_to(bias[None, :], (128, E)))

    in_maps = [None] * N_CORES

    def prep_core(c):
        xs = x[c * NSH : (c + 1) * NSH, :]
        xh = xs.astype(np.float16)
        xl = ((xs - xh.astype(np.float32)) * SPLIT_SCALE).astype(np.float16)
        in_maps[c] = {
            "xh": np.ascontiguousarray(xh.T),
            "xl": np.ascontiguousarray(xl.T),
            "w": w_packed,
            "bias": bias_rep,
        }

    threads = [threading.Thread(target=prep_core, args=(c,)) for c in range(N_CORES)]
    for th in threads:
        th.start()
    for th in threads:
        th.join()
    return in_maps


def kernel(x, weight, bias, _trace=False):
    if "nc" not in _cached:
        _cached["nc"] = _build_nc()
    nc = _cached["nc"]
    in_maps = _host_prep(x, weight, bias)
    res = run_bass_kernel_spmd(
        nc, in_maps, core_ids=list(range(N_CORES)), trace=_trace
    )
    _cached["last_result"] = res
    wts = np.concatenate([r["wts"] for r in res.results], axis=0)
    idx = np.concatenate([r["idx"] for r in res.results], axis=0)
    return wts, idx
